# revision 1
# baseline (speedup 1.0000x reference)
"""Trainium2 Bass kernel for nn_CharEncoder (bi-LSTM char encoder).

Strategy (8 NeuronCores, one SPMD program, per-core data):
  core c: dir = c//4 (0 = left LSTM, 1 = right LSTM), batch slice = c%4 (16 rows).
  Per core: gather embeddings (indirect DMA) -> PE-transpose to feature-major ->
  proj GEMM + tanh -> Wih GEMM (input-gate preactivations) to DRAM scratch ->
  256-step LSTM scan (Whh stationary tiles, bf16 matmuls, fp32 cell state).
Host side only reformats weights (transpose/permute/cast) and slices indices;
all model compute runs on device.

Gate-row permutation: the 16 row-chunks of Wih/Whh are reordered into two
halves (h-blocks {0,1} and {2,3}); within a half the slot order is
[i_b0, i_b1, f_b0, f_b1, o_b0, o_b1, g_b0, g_b1] so the scan's elementwise
work runs as a few large strided ops per half (i/f/o sigmoids in one slab).
"""
import sys

sys.path.insert(0, "/opt/trn_rl_repo")

import numpy as np
import ml_dtypes

import concourse.bass as bass
import concourse.bacc as bacc
import concourse.tile as tile
import concourse.mybir as mybir
from concourse.bass_utils import run_bass_kernel_spmd
from concourse.masks import make_identity

# Problem constants (hardcoded per harness contract).
VC, VB = 8000, 200000
DC = 100
E, H = 512, 512
B, S = 64, 256
P = 128
BL = B // 4          # local batch per core (4 batch slices x 2 dirs = 8 cores)
T = S * BL           # tokens per core = 4096
NJ = T // P          # 32 token tiles of 128
NT512 = T // 512     # 8 n-tiles of 512 tokens
JPN = 512 // P       # 4 token tiles per n-tile
KC = E // P          # 4 contraction chunks of 128
MC = (4 * H) // P    # 16 gate-row chunks of 128
F = 4 * DC           # 400 input features

DT_BF = mybir.dt.bfloat16
DT_F32 = mybir.dt.float32
NP_BF = ml_dtypes.bfloat16

AF = mybir.ActivationFunctionType

DEFAULT_REPS = {"pre": 1, "scan": 1, "amp": 0}  # timing builds: reps>1 or amp=R (HW loop)

_CACHE = {}


def _build_program(reps=None, opts=()):
    reps = dict(DEFAULT_REPS, **(reps or {}))
    opts = frozenset(opts)
    key = ("nc", opts) + tuple(sorted(reps.items()))
    if key in _CACHE:
        return _CACHE[key]

    nc = bacc.Bacc("TRN2", target_bir_lowering=False, debug=False, num_devices=8)

    def din(name, shape, dt):
        return nc.dram_tensor(name, shape, dt, kind="ExternalInput").ap()

    idxc = din("idxc", [P, NJ], mybir.dt.int32)
    idxb = din("idxb", [P, NJ], mybir.dt.int32)
    ctab = din("ctab", [VC, 2 * DC], DT_F32)      # [char_static | char] cols
    btab = din("btab", [VB, 2 * DC], DT_F32)      # [bichar_static | bichar] cols
    wt = din("wt", [F, E], DT_BF)                 # proj W.T
    pb = din("pb", [P, KC], DT_F32)               # proj bias chunks
    wiht = din("wiht", [E, 4 * H], DT_BF)         # Wih[perm].T
    whht = din("whht", [E, 4 * H], DT_BF)         # Whh[perm].T
    gb = din("gb", [P, MC], DT_F32)               # (bih+bhh)[perm] chunks
    out_ap = nc.dram_tensor("out", [S, P, KC, BL], DT_BF, kind="ExternalOutput").ap()

    with tile.TileContext(nc) as tc:
        with (
            tc.tile_pool(name="const", bufs=1) as cpool,
            tc.tile_pool(name="dram", bufs=1, space="DRAM") as dpool,
        ):
            ident = cpool.tile([P, P], DT_F32)
            make_identity(nc, ident[:])
            idxc_sb = cpool.tile([P, NJ], mybir.dt.int32)
            idxb_sb = cpool.tile([P, NJ], mybir.dt.int32)
            nc.sync.dma_start(out=idxc_sb[:], in_=idxc[:])
            nc.sync.dma_start(out=idxb_sb[:], in_=idxb[:])
            whht_sb = []
            for k in range(KC):
                w = cpool.tile([P, 4 * H], DT_BF, tag=f"whht{k}", name=f"whht{k}")
                nc.sync.dma_start(out=w[:], in_=whht[k * P:(k + 1) * P, :])
                whht_sb.append(w)
            pb_sb = cpool.tile([P, KC], DT_F32)
            gb_sb = cpool.tile([P, MC], DT_F32)
            nc.sync.dma_start(out=pb_sb[:], in_=pb[:])
            nc.sync.dma_start(out=gb_sb[:], in_=gb[:])
            # scan-read-optimal layout: per step one contiguous [P, MC*BL] slab
            wx_dram = dpool.tile([S, P, MC, BL], DT_F32)

            # ---- pre-scan: gather -> transpose -> proj -> Wx, pipelined per n-tile
            with (
                tc.tile_pool(name="mid", bufs=1) as mpool,
                tc.tile_pool(name="gath", bufs=8) as gpool,
                tc.tile_pool(name="xbuf", bufs=3) as xpool,
                tc.tile_pool(name="pst", bufs=2, space="PSUM") as pst,
                tc.tile_pool(name="psg", bufs=3, space="PSUM") as psg,
                tc.tile_pool(name="stage", bufs=4) as spool,
            ):
                wt_sb = []
                for k in range(KC):
                    kp = min(P, F - k * P)
                    w = mpool.tile([P, E], DT_BF, tag=f"wt{k}", name=f"wt{k}")
                    nc.sync.dma_start(out=w[:kp, :], in_=wt[k * P:k * P + kp, :])
                    wt_sb.append(w)
                wiht_sb = []
                for k in range(KC):
                    w = mpool.tile([P, 4 * H], DT_BF, tag=f"wiht{k}", name=f"wiht{k}")
                    nc.sync.dma_start(out=w[:], in_=wiht[k * P:(k + 1) * P, :])
                    wiht_sb.append(w)

                for _rp in range(reps["pre"]):
                    for nt in range(NT512):
                        xinT = [
                            xpool.tile([P, 512], DT_BF, tag=f"xinT{k}", name=f"xinT{k}")
                            for k in range(KC)
                        ]
                        for jj in range(JPN):
                            j = nt * JPN + jj
                            xg = gpool.tile([P, F], DT_F32, tag="xg")
                            nc.gpsimd.indirect_dma_start(
                                out=xg[:, 0:2 * DC], out_offset=None, in_=ctab[:],
                                in_offset=bass.IndirectOffsetOnAxis(
                                    ap=idxc_sb[:, j:j + 1], axis=0),
                            )
                            nc.gpsimd.indirect_dma_start(
                                out=xg[:, 2 * DC:F], out_offset=None, in_=btab[:],
                                in_offset=bass.IndirectOffsetOnAxis(
                                    ap=idxb_sb[:, j:j + 1], axis=0),
                            )
                            for fc in range(KC):
                                w = min(P, F - fc * P)
                                pt = pst.tile([P, P], DT_F32, tag="pt", space="PSUM")
                                nc.tensor.transpose(
                                    out=pt[:w, :], in_=xg[:, fc * P:fc * P + w],
                                    identity=ident[:])
                                nc.vector.tensor_copy(
                                    out=xinT[fc][:w, jj * P:(jj + 1) * P],
                                    in_=pt[:w, :])

                        # proj: xT_k = tanh(wt.T @ xinT + b) for this n-tile
                        xT = [
                            xpool.tile([P, 512], DT_BF, tag=f"xT{k}", name=f"xT{k}")
                            for k in range(KC)
                        ]
                        for m in range(KC):
                            ps = psg.tile([P, 512], DT_F32, tag="ps", name="psp",
                                          space="PSUM")
                            for k in range(KC):
                                kp = min(P, F - k * P)
                                nc.tensor.matmul(
                                    out=ps[:],
                                    lhsT=wt_sb[k][:kp, m * P:(m + 1) * P],
                                    rhs=xinT[k][:kp, :],
                                    start=(k == 0), stop=(k == KC - 1),
                                )
                            nc.scalar.activation(
                                out=xT[m][:], in_=ps[:], func=AF.Tanh,
                                bias=pb_sb[:, m:m + 1], scale=1.0)

                        # Wx: wiht.T @ xT + gbias -> wx_dram (step-major layout)
                        for m in range(MC):
                            ps = psg.tile([P, 512], DT_F32, tag="ps", name="psw",
                                          space="PSUM")
                            for k in range(KC):
                                nc.tensor.matmul(
                                    out=ps[:],
                                    lhsT=wiht_sb[k][:, m * P:(m + 1) * P],
                                    rhs=xT[k][:],
                                    start=(k == 0), stop=(k == KC - 1),
                                )
                            st = spool.tile([P, 512], DT_F32, tag="wxs")
                            nc.scalar.activation(
                                out=st[:], in_=ps[:], func=AF.Identity,
                                bias=gb_sb[:, m:m + 1], scale=1.0)
                            # tokens (s, b) of this n-tile -> wx_dram[s, :, m, :]
                            nc.sync.dma_start(
                                out=wx_dram[nt * 32:(nt + 1) * 32, :, m, :].rearrange(
                                    "s p b -> p s b"),
                                in_=st[:].rearrange("p (s b) -> p s b", b=BL),
                            )

            # ---- LSTM scan
            with (
                tc.tile_pool(name="scan_ps", bufs=2, space="PSUM") as sps,
                tc.tile_pool(name="state", bufs=3) as stp,
                tc.tile_pool(name="ew", bufs=4) as ewp,
                tc.tile_pool(name="wxp", bufs=6) as wxp,
            ):
                import contextlib
                _ampctx = (tc.For_i(0, reps["amp"], 1) if reps["amp"]
                           else contextlib.nullcontext())
                with _ampctx:
                  for _rs in range(reps["scan"]):
                    h_prev = stp.tile([P, KC, BL], DT_BF, tag="h")
                    c_prev = stp.tile([P, KC, BL], DT_F32, tag="c")
                    nc.vector.memset(h_prev[:], 0.0)
                    nc.vector.memset(c_prev[:], 0.0)

                    for t in range(S):
                        wx_t = wxp.tile([P, MC, BL], DT_F32, tag="wx")
                        nc.sync.dma_start(out=wx_t[:], in_=wx_dram[t])
                        h_new = stp.tile([P, KC, BL], DT_BF, tag="h")
                        c_new = stp.tile([P, KC, BL], DT_F32, tag="c")
                        for hh in range(2):
                            psh = sps.tile([P, 8, BL], DT_F32, tag=f"ps{hh}",
                                           name=f"ps{hh}", space="PSUM")
                            if "nomm" not in opts:
                              for slot in range(8):
                                m = 8 * hh + slot
                                for k in range(KC):
                                    nc.tensor.matmul(
                                        out=psh[:, slot, :],
                                        lhsT=whht_sb[k][:, m * P:(m + 1) * P],
                                        rhs=h_prev[:, k, :],
                                        start=(k == 0), stop=(k == KC - 1),
                                    )
                            elif hh == 0:
                                # touch psum so EW has defined-ish deps
                                nc.tensor.matmul(
                                    out=psh[:, 0, :], lhsT=whht_sb[0][:, 0:P],
                                    rhs=h_prev[:, 0, :], start=True, stop=True)
                            if "noew" in opts:
                                continue
                            # slots: [i0 i1 f0 f1 o0 o1 g0 g1] (blocks 2h, 2h+1)
                            bsl = slice(2 * hh, 2 * hh + 2)
                            pre = ewp.tile([P, 8, BL], DT_F32, tag="pre")
                            nc.vector.tensor_add(
                                out=pre[:], in0=psh[:],
                                in1=wx_t[:, 8 * hh:8 * hh + 8, :])
                            sact = ewp.tile([P, 6, BL], DT_F32, tag="sact")
                            nc.scalar.activation(
                                out=sact[:], in_=pre[:, 0:6, :], func=AF.Sigmoid)
                            gtan = ewp.tile([P, 2, BL], DT_F32, tag="gtan")
                            nc.scalar.activation(
                                out=gtan[:], in_=pre[:, 6:8, :], func=AF.Tanh)
                            t1 = ewp.tile([P, 2, BL], DT_F32, tag="t1")
                            t2 = ewp.tile([P, 2, BL], DT_F32, tag="t2")
                            nc.vector.tensor_mul(
                                out=t1[:], in0=sact[:, 2:4, :], in1=c_prev[:, bsl, :])
                            nc.vector.tensor_mul(
                                out=t2[:], in0=sact[:, 0:2, :], in1=gtan[:])
                            nc.vector.tensor_add(
                                out=c_new[:, bsl, :], in0=t1[:], in1=t2[:])
                            ctan = ewp.tile([P, 2, BL], DT_F32, tag="ctan")
                            nc.scalar.activation(
                                out=ctan[:], in_=c_new[:, bsl, :], func=AF.Tanh)
                            nc.vector.tensor_mul(
                                out=h_new[:, bsl, :], in0=sact[:, 4:6, :], in1=ctan[:])
                        if "noew" in opts:
                            nc.vector.tensor_copy(out=h_new[:], in_=h_prev[:])
                            nc.vector.tensor_copy(out=c_new[:], in_=c_prev[:])
                        nc.sync.dma_start(out=out_ap[t], in_=h_new[:])
                        h_prev, c_prev = h_new, c_new

    nc.compile()
    _CACHE[key] = nc
    return nc


def _gate_perm():
    # slot order per half: [i_b0 i_b1 f_b0 f_b1 o_b0 o_b1 g_b0 g_b1]
    # torch gate row-blocks: i=0, f=1, g=2, o=3
    rows = []
    for hh in range(2):
        for gate in (0, 1, 3, 2):
            for blk in (2 * hh, 2 * hh + 1):
                start = gate * H + blk * P
                rows.extend(range(start, start + P))
    return np.array(rows)


def _token_idx(insts_slice):
    # insts_slice [BL, S] -> [P, NJ] token-blocked (token t = s*BL + b)
    tok = np.arange(T)
    vals = insts_slice[tok % BL, tok // BL]        # [T]
    return np.ascontiguousarray(vals.reshape(NJ, P).T.astype(np.int32))


def _make_in_maps(inputs):
    f32 = np.float32
    ctab = np.ascontiguousarray(
        np.concatenate([inputs["char_tab_static"], inputs["char_tab"]], axis=1)
    ).astype(f32)
    btab = np.ascontiguousarray(
        np.concatenate([inputs["bichar_tab_static"], inputs["bichar_tab"]], axis=1)
    ).astype(f32)
    perm = _gate_perm()
    per_dir = []
    for d in range(2):
        sfx = "l" if d == 0 else "r"
        W = np.asarray(inputs[f"W_{sfx}"], f32)
        bvec = np.asarray(inputs[f"b_{sfx}"], f32)
        Wih = np.asarray(inputs[f"Wih_{sfx}"], f32)
        Whh = np.asarray(inputs[f"Whh_{sfx}"], f32)
        bsum = (np.asarray(inputs[f"bih_{sfx}"], f32)
                + np.asarray(inputs[f"bhh_{sfx}"], f32))
        per_dir.append({
            "wt": np.ascontiguousarray(W.T).astype(NP_BF),
            "pb": np.ascontiguousarray(bvec.reshape(KC, P).T).astype(f32),
            "wiht": np.ascontiguousarray(Wih[perm].T).astype(NP_BF),
            "whht": np.ascontiguousarray(Whh[perm].T).astype(NP_BF),
            "gb": np.ascontiguousarray(bsum[perm].reshape(MC, P).T).astype(f32),
        })
    in_maps = []
    for c in range(8):
        d, bs = divmod(c, 4)
        bsl = slice(BL * bs, BL * (bs + 1))
        m = {
            "idxc": _token_idx(np.asarray(inputs["insts_char"])[bsl]),
            "idxb": _token_idx(np.asarray(inputs["insts_bichar_l"])[bsl]),
            "ctab": ctab,
            "btab": btab,
        }
        m.update(per_dir[d])
        in_maps.append(m)
    return in_maps


def kernel(**inputs):
    nc = _build_program()
    in_maps = _make_in_maps(inputs)
    res = run_bass_kernel_spmd(nc, in_maps, core_ids=list(range(8)))
    full = np.zeros((S, B, 2 * H), dtype=np.float32)
    for c in range(8):
        d, bs = divmod(c, 4)
        r = np.asarray(res.results[c]["out"]).astype(np.float32)  # [S, P, KC, BL]
        r = r.transpose(0, 3, 2, 1).reshape(S, BL, H)
        full[:, BL * bs:BL * (bs + 1), H * d:H * (d + 1)] = r
    return full

